# revision 88
# baseline (speedup 1.0000x reference)
# Local (sliding-window, strictly-causal) multi-head attention for Trainium2.
#
# Problem: nn_LocalAttention  (B=2, S=4096, MD=AD=1024, NH=8, HD=128, window=256)
#   q = query @ Wq.T ; per-head scores q.k/sqrt(HD) masked to col in [row-256, row-1];
#   softmax; out = w @ v ; rows with no valid keys zeroed; out @ Wo.T.
#
# Sharding (8 cores): batch (2) x sequence chunks (4 x 1024 rows).  Each core runs
# the whole pipeline for its 1024 query rows using a 256-row K/V halo, so the 8
# output shards are disjoint and the gather is pure concatenation.  Weights are
# replicated and everything is cast to bf16 host-side (PSUM accumulation stays
# f32), which halves HBM traffic and runs the PE at full rate.
#
# Device pipeline per head (key-block-major, v-stationary):
#   scores[k,t] = kT_blk^T @ qT over the block's valid 128..384 query columns
#   (one matmul per key block), exp on ACT (no max subtraction: scores are O(1)
#   by construction), triangular window masks applied multiplicatively to the
#   <=2 edge pieces on DVE, then per-128-piece accumulation of both
#     out[d,t]   += v_blk^T @ e      (PSUM, 3 blocks per query tile)
#     denom[m,t] += ind_blk^T @ e    (PSUM, all-ones stationary per block)
#   The indicator stationary excludes zero-padded halo rows from the softmax
#   denominator (and seeds denom=1 for the no-valid-key row 0), absorbing all
#   per-core edge handling into data.  Output lands directly in [d,t] layout so
#   the Wo projection needs no transposes; normalization (reciprocal + scale)
#   happens on DVE during the PSUM->SBUF copy.

import math

import numpy as np
import ml_dtypes

import concourse.bass as bass
import concourse.tile as tile
from concourse import bacc, bass_isa, mybir
from concourse.bass_utils import run_bass_kernel_spmd

F32 = mybir.dt.float32
BF16 = mybir.dt.bfloat16
BF = ml_dtypes.bfloat16

NH = 8       # heads
HD = 128     # head dim
B = 2        # batch
S = 4096     # sequence
MD = 1024    # model dim
AD = 1024    # attn dim
WIN = 256    # window
C = 1024     # query rows per core (chunk)
NQT = C // 128          # 8 query tiles per chunk
HALO = WIN + C          # 1280 key/value rows per core
NKB = HALO // 128       # 10 key blocks
NCORES = 8


# ----------------------------------------------------------------------------
# device program
# ----------------------------------------------------------------------------

def _emit(ctx, tc: tile.TileContext, qcT, wqT, woT, kT, vb, pb, maskE, out):
    nc = tc.nc
    MUL = mybir.AluOpType.mult
    ADD = mybir.AluOpType.add

    wq_pool = ctx.enter_context(tc.tile_pool(name="wq", bufs=1))
    qc_pool = ctx.enter_context(tc.tile_pool(name="qc", bufs=1))
    kT_pool = ctx.enter_context(tc.tile_pool(name="kT", bufs=1))
    v_pool = ctx.enter_context(tc.tile_pool(name="v", bufs=1))
    pb_pool = ctx.enter_context(tc.tile_pool(name="pb", bufs=1))
    mask_pool = ctx.enter_context(tc.tile_pool(name="mask", bufs=1))
    wo_pool = ctx.enter_context(tc.tile_pool(name="wo", bufs=1))
    qT_pool = ctx.enter_context(tc.tile_pool(name="qT", bufs=1))
    att_pool = ctx.enter_context(tc.tile_pool(name="att", bufs=1))
    e_pool = ctx.enter_context(tc.tile_pool(name="e", bufs=7))
    stg_pool = ctx.enter_context(tc.tile_pool(name="stg", bufs=3))
    rc_pool = ctx.enter_context(tc.tile_pool(name="rc", bufs=2))
    stage_pool = ctx.enter_context(tc.tile_pool(name="stage", bufs=2))
    # PSUM: proj (4 banks) + scores (2) + out (2) = 8 banks
    proj_psum = ctx.enter_context(tc.tile_pool(name="proj", bufs=4, space="PSUM"))
    s_psum = ctx.enter_context(tc.tile_pool(name="s", bufs=2, space="PSUM"))
    od_psum = ctx.enter_context(tc.tile_pool(name="od", bufs=2, space="PSUM"))

    wq_sb = wq_pool.tile([128, NH, 8, 128], BF16)
    qc_sb = qc_pool.tile([128, 8, C], BF16)
    kT_sb = kT_pool.tile([128, NH, HALO], BF16)
    v_sb = v_pool.tile([128, NKB, AD], BF16)
    pb_sb = pb_pool.tile([128, NKB], F32)
    mask_sb = mask_pool.tile([128, 2, 128], BF16)
    wo_sb = wo_pool.tile([128, NH, MD], BF16)
    qT_sb = qT_pool.tile([128, NH, C], BF16)
    att_sb = att_pool.tile([128, NH, C], BF16)

    # DMA schedule.  In the cost model each issuing engine runs one transfer
    # at a time, so effective inbound bandwidth = #issuing engines; SP, ACT
    # and Pool (SWDGE) all carry streams, ordered by first use.  wq is
    # head-chunked (Qproj of head h only reads a 128-col slice of Wq, laid
    # out contiguously host-side), which unblocks the first Q projection
    # after ~0.5 MB instead of 4 MB of traffic.
    qc_r = qcT.rearrange("(mt p) t -> p mt t", p=128)
    kT_r = kT.rearrange("h d j -> d h j")
    v_r = vb.rearrange("b p a -> p b a")
    wo_r = woT.rearrange("(h d) o -> d h o", d=128)
    # SP: early qc chunks, then wq/v/kT tails ordered by first use
    nc.sync.dma_start(out=qc_sb[:, 0:2, :], in_=qc_r[:, 0:2, :])
    nc.sync.dma_start(out=qc_sb[:, 2:4, :], in_=qc_r[:, 2:4, :])
    nc.sync.dma_start(out=v_sb[:, 3:6, :], in_=v_r[:, 3:6, :])
    nc.sync.dma_start(out=wq_sb[:, 1, :, :], in_=wqT[:, 1, :, :])
    nc.sync.dma_start(out=v_sb[:, 6:10, :], in_=v_r[:, 6:10, :])
    nc.sync.dma_start(out=wq_sb[:, 2:8, :, :], in_=wqT[:, 2:8, :, :])
    nc.sync.dma_start(out=kT_sb[:, 4:8, :], in_=kT_r[:, 4:8, :])
    nc.sync.dma_start(out=wo_sb, in_=wo_r)
    # ACT: rest of qc, then free for exp
    nc.scalar.dma_start(out=qc_sb[:, 4:6, :], in_=qc_r[:, 4:6, :])
    nc.scalar.dma_start(out=qc_sb[:, 6:8, :], in_=qc_r[:, 6:8, :])
    # Pool: only the head-0 attention streams — it must be free for the
    # partition-reduce denominators by the time attention starts
    nc.gpsimd.dma_start(out=wq_sb[:, 0, :, :], in_=wqT[:, 0, :, :])
    nc.gpsimd.dma_start(out=kT_sb[:, 0, :], in_=kT_r[:, 0, :])
    nc.gpsimd.dma_start(out=mask_sb, in_=maskE)
    nc.gpsimd.dma_start(out=pb_sb, in_=pb)
    for blk in range(3):
        nc.gpsimd.dma_start(out=v_sb[:, blk, :], in_=v_r[:, blk, :])
    nc.gpsimd.dma_start(out=kT_sb[:, 1:4, :], in_=kT_r[:, 1:4, :])

    def _norm_chain(od, stg, h, half):
        # fold the 3 denominator partials, reciprocate, and scale the PSUM
        # output into att (all DVE).  bf16 partials/denominator round at
        # ~0.4% — far inside the rel-err budget — and run at the DVE's 2x
        # 2-byte rate.  Returned as four closures that the caller interleaves
        # with the NEXT half's mask ops: the DVE is strict-FIFO, so emitting
        # the whole chain at once would head-of-line-block those masks and
        # stall the PE's PV matmuls.
        box = {}

        def fold1():
            with nc.allow_low_precision(reason="bf16 softmax denominator"):
                box["den"] = rc_pool.tile([128, 512], BF16, name="den")
                nc.vector.tensor_tensor(
                    box["den"], stg[:, 0, :], stg[:, 1, :], ADD)

        def fold2():
            with nc.allow_low_precision(reason="bf16 softmax denominator"):
                nc.vector.tensor_tensor(
                    box["den"], box["den"], stg[:, 2, :], ADD)

        def recip():
            with nc.allow_low_precision(reason="bf16 softmax denominator"):
                box["rc"] = rc_pool.tile([128, 512], BF16, name="rc")
                nc.vector.reciprocal(box["rc"], box["den"])

        def mult():
            nc.vector.tensor_tensor(
                att_sb[:, h, half * 512:(half + 1) * 512], od, box["rc"], MUL)

        return [fold1, fold2, recip, mult]

    pending_norm = []
    for h in range(NH):
        # ---- Q projection for head h -> qT_sb[d, h, t] (bf16) ----
        # contraction chunks ordered by DMA arrival (SP: 01/23, ACT: 45/67)
        MT_ORDER = (0, 1, 4, 5, 2, 3, 6, 7)

        def _qproj_nn(qh, nn):
            pp = proj_psum.tile([128, 512], F32)
            for i, mt in enumerate(MT_ORDER):
                nc.tensor.matmul(
                    pp,
                    lhsT=wq_sb[:, qh, mt, :],
                    rhs=qc_sb[:, mt, nn * 512:(nn + 1) * 512],
                    start=(i == 0),
                    stop=(i == 7),
                )
            # one PSUM->SBUF copy per engine so neither FIFO eats both
            if nn == 0:
                nc.scalar.copy(qT_sb[:, qh, 0:512], pp)
            else:
                nc.vector.tensor_copy(qT_sb[:, qh, 512:1024], pp)

        for nn in (0, 1):
            _qproj_nn(h, nn)

        # ---- attention for head h, processed in two 512-column halves ----
        # Scores/exp are key-block-major; exp carries a per-partition pad
        # bias (-1e5 on zero-padded halo rows, plus the row-0 denom seed) so
        # padded keys vanish from both the output and the denominator.  The
        # softmax denominators are cross-partition sums done on the idle
        # GPSIMD engine (partition_all_reduce) into a [jj, t] staging buffer
        # (jj = 2-(kb-qt)), folded and reciprocated on DVE — no PE work.
        # The PSUM out accumulation is query-tile-major so each 128-col
        # piece's start..stop group is contiguous per bank.
        for half in range(2):
            q_lo = half * 4                          # first query tile of half
            od = od_psum.tile([128, 512], F32)       # unnormalized out[d, t]
            stg = stg_pool.tile([128, 3, 512], BF16)
            etiles = {}
            # producer runs 4 steps ahead of the PSUM consumer so the PE
            # never waits on the exp round-trip
            for step in range(7):
                kb = q_lo + step
                if pending_norm and step >= 3:
                    # one op of the previous half's normalize chain per step,
                    # interleaved between this half's masks in the DVE FIFO
                    pending_norm.pop(0)()
                if step < 6:
                    qts = [
                        qt for qt in range(max(kb - 2, 0), min(kb, NQT - 1) + 1)
                        if q_lo <= qt < q_lo + 4
                    ]
                    t0 = qts[0] * 128
                    w = 128 * len(qts)
                    sp = s_psum.tile([128, 384], F32)
                    nc.tensor.matmul(
                        sp[:, 0:w],
                        lhsT=kT_sb[:, h, kb * 128:(kb + 1) * 128],
                        rhs=qT_sb[:, h, t0:t0 + w],
                        start=True,
                        stop=True,
                    )
                    e = e_pool.tile([128, 384], BF16)
                    nc.scalar.activation(
                        e[:, 0:w], sp[:, 0:w],
                        mybir.ActivationFunctionType.Exp,
                        bias=pb_sb[:, kb:kb + 1],
                    )
                    # triangular window masks on edge pieces (in-place, DVE)
                    for qt in qts:
                        off = qt * 128 - t0
                        if qt == kb:        # newest keys: valid iff kk >= tt
                            nc.vector.tensor_tensor(
                                e[:, off:off + 128], e[:, off:off + 128],
                                mask_sb[:, 1, :], MUL)
                        elif qt == kb - 2:  # oldest keys: valid iff kk < tt
                            nc.vector.tensor_tensor(
                                e[:, off:off + 128], e[:, off:off + 128],
                                mask_sb[:, 0, :], MUL)
                    # denominator partials: sum e over partitions per piece
                    for qt in qts:
                        off = qt * 128 - t0
                        jj = 2 - (kb - qt)
                        col = qt * 128 - half * 512
                        nc.gpsimd.partition_all_reduce(
                            stg[:, jj, col:col + 128],
                            e[:, off:off + 128],
                            channels=128,
                            reduce_op=bass_isa.ReduceOp.add,
                        )
                    etiles[kb] = (e, t0)
                qt = q_lo + step - 3
                if q_lo <= qt < q_lo + 4:
                    col = qt * 128 - half * 512
                    for j, kbb in enumerate((qt, qt + 1, qt + 2)):
                        e, t0 = etiles[kbb]
                        off = qt * 128 - t0
                        nc.tensor.matmul(
                            od[:, col:col + 128],
                            lhsT=v_sb[:, kbb, h * 128:(h + 1) * 128],
                            rhs=e[:, off:off + 128],
                            start=(j == 0),
                            stop=(j == 2),
                        )
            pending_norm = _norm_chain(od, stg, h, half)
    for op in pending_norm:
        op()

    # ---- output projection: out[t, m] = sum_h att_h[:, t]^T @ Wo_h ----
    # the very last chunk is narrowed so the final copy+DMA drain is short
    for qt in range(NQT):
        chunks = ((0, 512), (512, 1024)) if qt < NQT - 1 else \
                 ((0, 512), (512, 768), (768, 1024))
        for m0, m1 in chunks:
            pp = proj_psum.tile([128, 512], F32)
            for hh in range(NH):
                nc.tensor.matmul(
                    pp[:, 0:m1 - m0],
                    lhsT=att_sb[:, hh, qt * 128:(qt + 1) * 128],
                    rhs=wo_sb[:, hh, m0:m1],
                    start=(hh == 0),
                    stop=(hh == NH - 1),
                )
            st = stage_pool.tile([128, 512], BF16)
            nc.scalar.copy(st[:, 0:m1 - m0], pp[:, 0:m1 - m0])
            nc.sync.dma_start(
                out=out[qt * 128:(qt + 1) * 128, m0:m1],
                in_=st[:, 0:m1 - m0],
            )


_CACHED_NC = {}


def _build_program(iters: int = 1):
    if iters in _CACHED_NC:
        return _CACHED_NC[iters]
    nc = bacc.Bacc("TRN2", target_bir_lowering=False, debug=False)
    qcT = nc.dram_tensor("qcT", [MD, C], BF16, kind="ExternalInput").ap()
    # wqT is pre-laid-out host-side in device order: [p, head, mt, a]
    wqT = nc.dram_tensor("wqT", [128, NH, 8, 128], BF16, kind="ExternalInput").ap()
    woT = nc.dram_tensor("woT", [AD, MD], BF16, kind="ExternalInput").ap()
    kT = nc.dram_tensor("kT", [NH, HD, HALO], BF16, kind="ExternalInput").ap()
    vb = nc.dram_tensor("vb", [NKB, 128, AD], BF16, kind="ExternalInput").ap()
    pb = nc.dram_tensor("pb", [128, NKB], F32, kind="ExternalInput").ap()
    maskE = nc.dram_tensor("maskE", [128, 2, 128], BF16, kind="ExternalInput").ap()
    out = nc.dram_tensor("out", [C, MD], BF16, kind="ExternalOutput").ap()
    from contextlib import ExitStack

    with tile.TileContext(nc) as tc:
        for _ in range(iters):
            with ExitStack() as ctx:
                _emit(ctx, tc, qcT, wqT, woT, kT, vb, pb, maskE, out)
    nc.compile()
    _CACHED_NC[iters] = nc
    return nc


# ----------------------------------------------------------------------------
# host-side shard construction
# ----------------------------------------------------------------------------

def _make_in_maps(query_seq, keys_seq, values_seq, Wq, Wo):
    q = np.asarray(query_seq, dtype=np.float32)
    k = np.asarray(keys_seq, dtype=np.float32)
    v = np.asarray(values_seq, dtype=np.float32)
    wq = np.asarray(Wq, dtype=np.float32)
    wo = np.asarray(Wo, dtype=np.float32)

    scale = np.float32(math.sqrt(float(HD)))
    # [m, a] -> device layout [p, head, mt, a128]  (m = mt*128+p, a = h*128+aa)
    wqT = np.ascontiguousarray(
        (wq.T / scale).reshape(8, 128, NH, 128).transpose(1, 2, 0, 3)
    ).astype(BF)
    woT = np.ascontiguousarray(wo.T).astype(BF)

    # triangular window masks, shared by all cores: [kk, {sub2, sub0}, tt]
    kk = np.arange(128)[:, None]
    tt = np.arange(128)[None, :]
    maskE = np.zeros((128, 2, 128), BF)
    maskE[:, 0, :] = (kk < tt).astype(BF)   # oldest-keys piece
    maskE[:, 1, :] = (kk >= tt).astype(BF)  # newest-keys piece

    in_maps = []
    for core in range(NCORES):
        b, ch = divmod(core, S // C)
        s0 = ch * C

        qcT = np.ascontiguousarray(q[b, s0:s0 + C, :].T).astype(BF)   # [MD, C]

        khalo = np.zeros((HALO, AD), np.float32)
        vhalo = np.zeros((HALO, AD), np.float32)
        lo = s0 - WIN
        off = max(0, -lo)
        khalo[off:] = k[b, lo + off:s0 + C, :]
        vhalo[off:] = v[b, lo + off:s0 + C, :]

        kT = np.ascontiguousarray(
            khalo.reshape(HALO, NH, HD).transpose(1, 2, 0)).astype(BF)
        vb = np.ascontiguousarray(vhalo.reshape(NKB, 128, AD)).astype(BF)

        # per-partition exp bias: -1e5 on zero-padded halo rows (removes them
        # from both the output and the denominator), 0 elsewhere; a single
        # seed entry gives the no-valid-key row 0 denom=1 -> out=0.
        pb = np.zeros((128, NKB), np.float32)
        for blk in range(NKB):
            k_abs = s0 - WIN + blk * 128 + np.arange(128)
            pb[:, blk] = np.where(k_abs >= 0, 0.0, -1.0e5)
        if s0 == 0:
            pb[0, 0] = 0.0
        pb = np.ascontiguousarray(pb)

        in_maps.append({
            "qcT": qcT,
            "wqT": wqT,
            "woT": woT,
            "kT": kT,
            "vb": vb,
            "pb": pb,
            "maskE": maskE,
        })
    return in_maps


def _gather(results) -> np.ndarray:
    out = np.empty((B, S, MD), np.float32)
    for core in range(NCORES):
        b, ch = divmod(core, S // C)
        out[b, ch * C:(ch + 1) * C, :] = results[core]["out"].astype(np.float32)
    return out


def _run(in_maps, **kwargs):
    nc = _build_program()
    return run_bass_kernel_spmd(nc, in_maps, list(range(NCORES)), **kwargs)


def kernel(query_seq, keys_seq, values_seq, Wq, Wo, window=WIN, **_unused):
    assert int(window) == WIN, f"kernel hardcodes window={WIN}, got {window}"
    in_maps = _make_in_maps(query_seq, keys_seq, values_seq, Wq, Wo)
    res = _run(in_maps)
    return _gather(res.results)


def kernel_traced(query_seq, keys_seq, values_seq, Wq, Wo, window=WIN, **_unused):
    """Like kernel() but also returns BassKernelResults (profile/exec time)."""
    assert int(window) == WIN
    in_maps = _make_in_maps(query_seq, keys_seq, values_seq, Wq, Wo)
    res = _run(in_maps, trace=True)
    return _gather(res.results), res


# revision 92
# speedup vs baseline: 1.0016x; 1.0016x over previous
# Local (sliding-window, strictly-causal) multi-head attention for Trainium2.
#
# Problem: nn_LocalAttention  (B=2, S=4096, MD=AD=1024, NH=8, HD=128, window=256)
#   q = query @ Wq.T ; per-head scores q.k/sqrt(HD) masked to col in [row-256, row-1];
#   softmax; out = w @ v ; rows with no valid keys zeroed; out @ Wo.T.
#
# Sharding (8 cores): batch (2) x sequence chunks (4 x 1024 rows).  Each core runs
# the whole pipeline for its 1024 query rows using a 256-row K/V halo, so the 8
# output shards are disjoint and the gather is pure concatenation.  Weights are
# replicated and everything is cast to bf16 host-side (PSUM accumulation stays
# f32), which halves HBM traffic and runs the PE at full rate.
#
# Device pipeline per head (key-block-major, v-stationary):
#   scores[k,t] = kT_blk^T @ qT over the block's valid 128..384 query columns
#   (one matmul per key block), exp on ACT (no max subtraction: scores are O(1)
#   by construction), triangular window masks applied multiplicatively to the
#   <=2 edge pieces on DVE, then per-128-piece accumulation of both
#     out[d,t]   += v_blk^T @ e      (PSUM, 3 blocks per query tile)
#     denom[m,t] += ind_blk^T @ e    (PSUM, all-ones stationary per block)
#   The indicator stationary excludes zero-padded halo rows from the softmax
#   denominator (and seeds denom=1 for the no-valid-key row 0), absorbing all
#   per-core edge handling into data.  Output lands directly in [d,t] layout so
#   the Wo projection needs no transposes; normalization (reciprocal + scale)
#   happens on DVE during the PSUM->SBUF copy.

import math

import numpy as np
import ml_dtypes

import concourse.bass as bass
import concourse.tile as tile
from concourse import bacc, bass_isa, mybir
from concourse.bass_utils import run_bass_kernel_spmd

F32 = mybir.dt.float32
BF16 = mybir.dt.bfloat16
BF = ml_dtypes.bfloat16

NH = 8       # heads
HD = 128     # head dim
B = 2        # batch
S = 4096     # sequence
MD = 1024    # model dim
AD = 1024    # attn dim
WIN = 256    # window
C = 1024     # query rows per core (chunk)
NQT = C // 128          # 8 query tiles per chunk
HALO = WIN + C          # 1280 key/value rows per core
NKB = HALO // 128       # 10 key blocks
NCORES = 8


# ----------------------------------------------------------------------------
# device program
# ----------------------------------------------------------------------------

def _emit(ctx, tc: tile.TileContext, qcT, wqT, woT, kT, vb, pb, maskE, out):
    nc = tc.nc
    MUL = mybir.AluOpType.mult
    ADD = mybir.AluOpType.add

    wq_pool = ctx.enter_context(tc.tile_pool(name="wq", bufs=1))
    qc_pool = ctx.enter_context(tc.tile_pool(name="qc", bufs=1))
    kT_pool = ctx.enter_context(tc.tile_pool(name="kT", bufs=1))
    v_pool = ctx.enter_context(tc.tile_pool(name="v", bufs=1))
    pb_pool = ctx.enter_context(tc.tile_pool(name="pb", bufs=1))
    mask_pool = ctx.enter_context(tc.tile_pool(name="mask", bufs=1))
    wo_pool = ctx.enter_context(tc.tile_pool(name="wo", bufs=1))
    qT_pool = ctx.enter_context(tc.tile_pool(name="qT", bufs=1))
    att_pool = ctx.enter_context(tc.tile_pool(name="att", bufs=1))
    e_pool = ctx.enter_context(tc.tile_pool(name="e", bufs=7))
    stg_pool = ctx.enter_context(tc.tile_pool(name="stg", bufs=3))
    rc_pool = ctx.enter_context(tc.tile_pool(name="rc", bufs=2))
    stage_pool = ctx.enter_context(tc.tile_pool(name="stage", bufs=2))
    # PSUM: proj (4 banks) + scores (2) + out (2) = 8 banks
    proj_psum = ctx.enter_context(tc.tile_pool(name="proj", bufs=4, space="PSUM"))
    s_psum = ctx.enter_context(tc.tile_pool(name="s", bufs=2, space="PSUM"))
    od_psum = ctx.enter_context(tc.tile_pool(name="od", bufs=2, space="PSUM"))

    wq_sb = wq_pool.tile([128, NH, 8, 128], BF16)
    qc_sb = qc_pool.tile([128, 8, C], BF16)
    kT_sb = kT_pool.tile([128, NH, HALO], BF16)
    v_sb = v_pool.tile([128, NKB, AD], BF16)
    pb_sb = pb_pool.tile([128, NKB], F32)
    mask_sb = mask_pool.tile([128, 2, 128], BF16)
    wo_sb = wo_pool.tile([128, NH, MD], BF16)
    qT_sb = qT_pool.tile([128, NH, C], BF16)
    att_sb = att_pool.tile([128, NH, C], BF16)

    # DMA schedule.  In the cost model each issuing engine runs one transfer
    # at a time, so effective inbound bandwidth = #issuing engines; SP, ACT
    # and Pool (SWDGE) all carry streams, ordered by first use.  wq is
    # head-chunked (Qproj of head h only reads a 128-col slice of Wq, laid
    # out contiguously host-side), which unblocks the first Q projection
    # after ~0.5 MB instead of 4 MB of traffic.
    qc_r = qcT.rearrange("(mt p) t -> p mt t", p=128)
    kT_r = kT.rearrange("h d j -> d h j")
    v_r = vb.rearrange("b p a -> p b a")
    wo_r = woT.rearrange("(h d) o -> d h o", d=128)
    # SP: early qc chunks, then wq/v/kT tails ordered by first use
    nc.sync.dma_start(out=qc_sb[:, 0, :], in_=qc_r[:, 0, :])
    nc.sync.dma_start(out=qc_sb[:, 1, :], in_=qc_r[:, 1, :])
    nc.sync.dma_start(out=qc_sb[:, 2:4, :], in_=qc_r[:, 2:4, :])
    nc.sync.dma_start(out=v_sb[:, 3:6, :], in_=v_r[:, 3:6, :])
    nc.sync.dma_start(out=wq_sb[:, 1, :, :], in_=wqT[:, 1, :, :])
    nc.sync.dma_start(out=v_sb[:, 6:10, :], in_=v_r[:, 6:10, :])
    nc.sync.dma_start(out=wq_sb[:, 2:8, :, :], in_=wqT[:, 2:8, :, :])
    nc.sync.dma_start(out=kT_sb[:, 4:8, :], in_=kT_r[:, 4:8, :])
    nc.sync.dma_start(out=wo_sb, in_=wo_r)
    # ACT: rest of qc, then free for exp
    nc.scalar.dma_start(out=qc_sb[:, 4:6, :], in_=qc_r[:, 4:6, :])
    nc.scalar.dma_start(out=qc_sb[:, 6:8, :], in_=qc_r[:, 6:8, :])
    # Pool: only the head-0 attention streams — it must be free for the
    # partition-reduce denominators by the time attention starts
    nc.gpsimd.dma_start(out=wq_sb[:, 0, :, :], in_=wqT[:, 0, :, :])
    nc.gpsimd.dma_start(out=kT_sb[:, 0, :], in_=kT_r[:, 0, :])
    nc.gpsimd.dma_start(out=mask_sb, in_=maskE)
    nc.gpsimd.dma_start(out=pb_sb, in_=pb)
    for blk in range(3):
        nc.gpsimd.dma_start(out=v_sb[:, blk, :], in_=v_r[:, blk, :])
    nc.gpsimd.dma_start(out=kT_sb[:, 1:4, :], in_=kT_r[:, 1:4, :])

    def _norm_chain(od, stg, h, half):
        # fold the 3 denominator partials, reciprocate, and scale the PSUM
        # output into att (all DVE).  bf16 partials/denominator round at
        # ~0.4% — far inside the rel-err budget — and run at the DVE's 2x
        # 2-byte rate.  Returned as four closures that the caller interleaves
        # with the NEXT half's mask ops: the DVE is strict-FIFO, so emitting
        # the whole chain at once would head-of-line-block those masks and
        # stall the PE's PV matmuls.
        box = {}

        def fold1():
            with nc.allow_low_precision(reason="bf16 softmax denominator"):
                box["den"] = rc_pool.tile([128, 512], BF16, name="den")
                nc.vector.tensor_tensor(
                    box["den"], stg[:, 0, :], stg[:, 1, :], ADD)

        def fold2():
            with nc.allow_low_precision(reason="bf16 softmax denominator"):
                nc.vector.tensor_tensor(
                    box["den"], box["den"], stg[:, 2, :], ADD)

        def recip():
            with nc.allow_low_precision(reason="bf16 softmax denominator"):
                box["rc"] = rc_pool.tile([128, 512], BF16, name="rc")
                nc.vector.reciprocal(box["rc"], box["den"])

        def mult():
            nc.vector.tensor_tensor(
                att_sb[:, h, half * 512:(half + 1) * 512], od, box["rc"], MUL)

        return [fold1, fold2, recip, mult]

    pending_norm = []
    for h in range(NH):
        # ---- Q projection for head h -> qT_sb[d, h, t] (bf16) ----
        # contraction chunks ordered by DMA arrival (SP: 01/23, ACT: 45/67)
        MT_ORDER = (0, 1, 4, 5, 2, 3, 6, 7)

        def _qproj_nn(qh, nn):
            pp = proj_psum.tile([128, 512], F32)
            for i, mt in enumerate(MT_ORDER):
                nc.tensor.matmul(
                    pp,
                    lhsT=wq_sb[:, qh, mt, :],
                    rhs=qc_sb[:, mt, nn * 512:(nn + 1) * 512],
                    start=(i == 0),
                    stop=(i == 7),
                )
            # one PSUM->SBUF copy per engine so neither FIFO eats both
            if nn == 0:
                nc.scalar.copy(qT_sb[:, qh, 0:512], pp)
            else:
                nc.vector.tensor_copy(qT_sb[:, qh, 512:1024], pp)

        for nn in (0, 1):
            _qproj_nn(h, nn)

        # ---- attention for head h, processed in two 512-column halves ----
        # Scores/exp are key-block-major; exp carries a per-partition pad
        # bias (-1e5 on zero-padded halo rows, plus the row-0 denom seed) so
        # padded keys vanish from both the output and the denominator.  The
        # softmax denominators are cross-partition sums done on the idle
        # GPSIMD engine (partition_all_reduce) into a [jj, t] staging buffer
        # (jj = 2-(kb-qt)), folded and reciprocated on DVE — no PE work.
        # The PSUM out accumulation is query-tile-major so each 128-col
        # piece's start..stop group is contiguous per bank.
        for half in range(2):
            q_lo = half * 4                          # first query tile of half
            od = od_psum.tile([128, 512], F32)       # unnormalized out[d, t]
            stg = stg_pool.tile([128, 3, 512], BF16)
            etiles = {}
            # producer runs 4 steps ahead of the PSUM consumer so the PE
            # never waits on the exp round-trip
            for step in range(7):
                kb = q_lo + step
                if pending_norm and step >= 3:
                    # one op of the previous half's normalize chain per step,
                    # interleaved between this half's masks in the DVE FIFO
                    pending_norm.pop(0)()
                if step < 6:
                    qts = [
                        qt for qt in range(max(kb - 2, 0), min(kb, NQT - 1) + 1)
                        if q_lo <= qt < q_lo + 4
                    ]
                    t0 = qts[0] * 128
                    w = 128 * len(qts)
                    sp = s_psum.tile([128, 384], F32)
                    nc.tensor.matmul(
                        sp[:, 0:w],
                        lhsT=kT_sb[:, h, kb * 128:(kb + 1) * 128],
                        rhs=qT_sb[:, h, t0:t0 + w],
                        start=True,
                        stop=True,
                    )
                    e = e_pool.tile([128, 384], BF16)
                    nc.scalar.activation(
                        e[:, 0:w], sp[:, 0:w],
                        mybir.ActivationFunctionType.Exp,
                        bias=pb_sb[:, kb:kb + 1],
                    )
                    # triangular window masks on edge pieces (in-place, DVE)
                    for qt in qts:
                        off = qt * 128 - t0
                        if qt == kb:        # newest keys: valid iff kk >= tt
                            nc.vector.tensor_tensor(
                                e[:, off:off + 128], e[:, off:off + 128],
                                mask_sb[:, 1, :], MUL)
                        elif qt == kb - 2:  # oldest keys: valid iff kk < tt
                            nc.vector.tensor_tensor(
                                e[:, off:off + 128], e[:, off:off + 128],
                                mask_sb[:, 0, :], MUL)
                    # denominator partials: sum e over partitions per piece
                    for qt in qts:
                        off = qt * 128 - t0
                        jj = 2 - (kb - qt)
                        col = qt * 128 - half * 512
                        nc.gpsimd.partition_all_reduce(
                            stg[:, jj, col:col + 128],
                            e[:, off:off + 128],
                            channels=128,
                            reduce_op=bass_isa.ReduceOp.add,
                        )
                    etiles[kb] = (e, t0)
                qt = q_lo + step - 3
                if q_lo <= qt < q_lo + 4:
                    col = qt * 128 - half * 512
                    for j, kbb in enumerate((qt, qt + 1, qt + 2)):
                        e, t0 = etiles[kbb]
                        off = qt * 128 - t0
                        nc.tensor.matmul(
                            od[:, col:col + 128],
                            lhsT=v_sb[:, kbb, h * 128:(h + 1) * 128],
                            rhs=e[:, off:off + 128],
                            start=(j == 0),
                            stop=(j == 2),
                        )
            pending_norm = _norm_chain(od, stg, h, half)
    for op in pending_norm:
        op()

    # ---- output projection: out[t, m] = sum_h att_h[:, t]^T @ Wo_h ----
    # the very last chunk is narrowed so the final copy+DMA drain is short
    for qt in range(NQT):
        chunks = ((0, 512), (512, 1024)) if qt < NQT - 1 else \
                 ((0, 512), (512, 768), (768, 1024))
        for m0, m1 in chunks:
            pp = proj_psum.tile([128, 512], F32)
            for hh in range(NH):
                nc.tensor.matmul(
                    pp[:, 0:m1 - m0],
                    lhsT=att_sb[:, hh, qt * 128:(qt + 1) * 128],
                    rhs=wo_sb[:, hh, m0:m1],
                    start=(hh == 0),
                    stop=(hh == NH - 1),
                )
            st = stage_pool.tile([128, 512], BF16)
            nc.scalar.copy(st[:, 0:m1 - m0], pp[:, 0:m1 - m0])
            nc.sync.dma_start(
                out=out[qt * 128:(qt + 1) * 128, m0:m1],
                in_=st[:, 0:m1 - m0],
            )


_CACHED_NC = {}


def _build_program(iters: int = 1):
    if iters in _CACHED_NC:
        return _CACHED_NC[iters]
    nc = bacc.Bacc("TRN2", target_bir_lowering=False, debug=False)
    qcT = nc.dram_tensor("qcT", [MD, C], BF16, kind="ExternalInput").ap()
    # wqT is pre-laid-out host-side in device order: [p, head, mt, a]
    wqT = nc.dram_tensor("wqT", [128, NH, 8, 128], BF16, kind="ExternalInput").ap()
    woT = nc.dram_tensor("woT", [AD, MD], BF16, kind="ExternalInput").ap()
    kT = nc.dram_tensor("kT", [NH, HD, HALO], BF16, kind="ExternalInput").ap()
    vb = nc.dram_tensor("vb", [NKB, 128, AD], BF16, kind="ExternalInput").ap()
    pb = nc.dram_tensor("pb", [128, NKB], F32, kind="ExternalInput").ap()
    maskE = nc.dram_tensor("maskE", [128, 2, 128], BF16, kind="ExternalInput").ap()
    out = nc.dram_tensor("out", [C, MD], BF16, kind="ExternalOutput").ap()
    from contextlib import ExitStack

    with tile.TileContext(nc) as tc:
        for _ in range(iters):
            with ExitStack() as ctx:
                _emit(ctx, tc, qcT, wqT, woT, kT, vb, pb, maskE, out)
    nc.compile()
    _CACHED_NC[iters] = nc
    return nc


# ----------------------------------------------------------------------------
# host-side shard construction
# ----------------------------------------------------------------------------

def _make_in_maps(query_seq, keys_seq, values_seq, Wq, Wo):
    q = np.asarray(query_seq, dtype=np.float32)
    k = np.asarray(keys_seq, dtype=np.float32)
    v = np.asarray(values_seq, dtype=np.float32)
    wq = np.asarray(Wq, dtype=np.float32)
    wo = np.asarray(Wo, dtype=np.float32)

    scale = np.float32(math.sqrt(float(HD)))
    # [m, a] -> device layout [p, head, mt, a128]  (m = mt*128+p, a = h*128+aa)
    wqT = np.ascontiguousarray(
        (wq.T / scale).reshape(8, 128, NH, 128).transpose(1, 2, 0, 3)
    ).astype(BF)
    woT = np.ascontiguousarray(wo.T).astype(BF)

    # triangular window masks, shared by all cores: [kk, {sub2, sub0}, tt]
    kk = np.arange(128)[:, None]
    tt = np.arange(128)[None, :]
    maskE = np.zeros((128, 2, 128), BF)
    maskE[:, 0, :] = (kk < tt).astype(BF)   # oldest-keys piece
    maskE[:, 1, :] = (kk >= tt).astype(BF)  # newest-keys piece

    in_maps = []
    for core in range(NCORES):
        b, ch = divmod(core, S // C)
        s0 = ch * C

        qcT = np.ascontiguousarray(q[b, s0:s0 + C, :].T).astype(BF)   # [MD, C]

        khalo = np.zeros((HALO, AD), np.float32)
        vhalo = np.zeros((HALO, AD), np.float32)
        lo = s0 - WIN
        off = max(0, -lo)
        khalo[off:] = k[b, lo + off:s0 + C, :]
        vhalo[off:] = v[b, lo + off:s0 + C, :]

        kT = np.ascontiguousarray(
            khalo.reshape(HALO, NH, HD).transpose(1, 2, 0)).astype(BF)
        vb = np.ascontiguousarray(vhalo.reshape(NKB, 128, AD)).astype(BF)

        # per-partition exp bias: -1e5 on zero-padded halo rows (removes them
        # from both the output and the denominator), 0 elsewhere; a single
        # seed entry gives the no-valid-key row 0 denom=1 -> out=0.
        pb = np.zeros((128, NKB), np.float32)
        for blk in range(NKB):
            k_abs = s0 - WIN + blk * 128 + np.arange(128)
            pb[:, blk] = np.where(k_abs >= 0, 0.0, -1.0e5)
        if s0 == 0:
            pb[0, 0] = 0.0
        pb = np.ascontiguousarray(pb)

        in_maps.append({
            "qcT": qcT,
            "wqT": wqT,
            "woT": woT,
            "kT": kT,
            "vb": vb,
            "pb": pb,
            "maskE": maskE,
        })
    return in_maps


def _gather(results) -> np.ndarray:
    out = np.empty((B, S, MD), np.float32)
    for core in range(NCORES):
        b, ch = divmod(core, S // C)
        out[b, ch * C:(ch + 1) * C, :] = results[core]["out"].astype(np.float32)
    return out


def _run(in_maps, **kwargs):
    nc = _build_program()
    return run_bass_kernel_spmd(nc, in_maps, list(range(NCORES)), **kwargs)


def kernel(query_seq, keys_seq, values_seq, Wq, Wo, window=WIN, **_unused):
    assert int(window) == WIN, f"kernel hardcodes window={WIN}, got {window}"
    in_maps = _make_in_maps(query_seq, keys_seq, values_seq, Wq, Wo)
    res = _run(in_maps)
    return _gather(res.results)


def kernel_traced(query_seq, keys_seq, values_seq, Wq, Wo, window=WIN, **_unused):
    """Like kernel() but also returns BassKernelResults (profile/exec time)."""
    assert int(window) == WIN
    in_maps = _make_in_maps(query_seq, keys_seq, values_seq, Wq, Wo)
    res = _run(in_maps, trace=True)
    return _gather(res.results), res


# revision 97
# speedup vs baseline: 1.0253x; 1.0237x over previous
# Local (sliding-window, strictly-causal) multi-head attention for Trainium2.
#
# Problem: nn_LocalAttention  (B=2, S=4096, MD=AD=1024, NH=8, HD=128, window=256)
#   q = query @ Wq.T ; per-head scores q.k/sqrt(HD) masked to col in [row-256, row-1];
#   softmax; out = w @ v ; rows with no valid keys zeroed; out @ Wo.T.
#
# Sharding (8 cores): batch (2) x sequence chunks (4 x 1024 rows).  Each core runs
# the whole pipeline for its 1024 query rows using a 256-row K/V halo, so the 8
# output shards are disjoint and the gather is pure concatenation.  Weights are
# replicated and everything is cast to bf16 host-side (PSUM accumulation stays
# f32), which halves HBM traffic and runs the PE at full rate.
#
# Device pipeline per head (key-block-major, v-stationary):
#   scores[k,t] = kT_blk^T @ qT over the block's valid 128..384 query columns
#   (one matmul per key block), exp on ACT (no max subtraction: scores are O(1)
#   by construction), triangular window masks applied multiplicatively to the
#   <=2 edge pieces on DVE, then per-128-piece accumulation of both
#     out[d,t]   += v_blk^T @ e      (PSUM, 3 blocks per query tile)
#     denom[m,t] += ind_blk^T @ e    (PSUM, all-ones stationary per block)
#   The indicator stationary excludes zero-padded halo rows from the softmax
#   denominator (and seeds denom=1 for the no-valid-key row 0), absorbing all
#   per-core edge handling into data.  Output lands directly in [d,t] layout so
#   the Wo projection needs no transposes; normalization (reciprocal + scale)
#   happens on DVE during the PSUM->SBUF copy.

import math

import numpy as np
import ml_dtypes

import concourse.bass as bass
import concourse.tile as tile
from concourse import bacc, bass_isa, mybir
from concourse.bass_utils import run_bass_kernel_spmd

F32 = mybir.dt.float32
BF16 = mybir.dt.bfloat16
BF = ml_dtypes.bfloat16

NH = 8       # heads
HD = 128     # head dim
B = 2        # batch
S = 4096     # sequence
MD = 1024    # model dim
AD = 1024    # attn dim
WIN = 256    # window
C = 1024     # query rows per core (chunk)
NQT = C // 128          # 8 query tiles per chunk
HALO = WIN + C          # 1280 key/value rows per core
NKB = HALO // 128       # 10 key blocks
NCORES = 8


# ----------------------------------------------------------------------------
# device program
# ----------------------------------------------------------------------------

def _emit(ctx, tc: tile.TileContext, qcT, wqT, woT, kT, vb, pb, maskE, out):
    nc = tc.nc
    MUL = mybir.AluOpType.mult
    ADD = mybir.AluOpType.add

    wq_pool = ctx.enter_context(tc.tile_pool(name="wq", bufs=1))
    qc_pool = ctx.enter_context(tc.tile_pool(name="qc", bufs=1))
    kT_pool = ctx.enter_context(tc.tile_pool(name="kT", bufs=1))
    v_pool = ctx.enter_context(tc.tile_pool(name="v", bufs=1))
    pb_pool = ctx.enter_context(tc.tile_pool(name="pb", bufs=1))
    mask_pool = ctx.enter_context(tc.tile_pool(name="mask", bufs=1))
    wo_pool = ctx.enter_context(tc.tile_pool(name="wo", bufs=1))
    qT_pool = ctx.enter_context(tc.tile_pool(name="qT", bufs=1))
    att_pool = ctx.enter_context(tc.tile_pool(name="att", bufs=1))
    e_pool = ctx.enter_context(tc.tile_pool(name="e", bufs=7))
    stg_pool = ctx.enter_context(tc.tile_pool(name="stg", bufs=3))
    rc_pool = ctx.enter_context(tc.tile_pool(name="rc", bufs=2))
    stage_pool = ctx.enter_context(tc.tile_pool(name="stage", bufs=2))
    # PSUM: proj (4 banks) + scores (2) + out (2) = 8 banks
    proj_psum = ctx.enter_context(tc.tile_pool(name="proj", bufs=4, space="PSUM"))
    s_psum = ctx.enter_context(tc.tile_pool(name="s", bufs=2, space="PSUM"))
    od_psum = ctx.enter_context(tc.tile_pool(name="od", bufs=2, space="PSUM"))

    wq_sb = wq_pool.tile([128, NH, 8, 128], BF16)
    qc_sb = qc_pool.tile([128, 8, C], BF16)
    kT_sb = kT_pool.tile([128, NH, HALO], BF16)
    v_sb = v_pool.tile([128, NKB, AD], BF16)
    pb_sb = pb_pool.tile([128, NKB], F32)
    mask_sb = mask_pool.tile([128, 2, 128], BF16)
    wo_sb = wo_pool.tile([128, NH, MD], BF16)
    qT_sb = qT_pool.tile([128, NH, C], BF16)
    att_sb = att_pool.tile([128, NH, C], BF16)

    # DMA schedule.  In the cost model each issuing engine runs one transfer
    # at a time, so effective inbound bandwidth = #issuing engines; SP, ACT
    # and Pool (SWDGE) all carry streams, ordered by first use.  wq is
    # head-chunked (Qproj of head h only reads a 128-col slice of Wq, laid
    # out contiguously host-side), which unblocks the first Q projection
    # after ~0.5 MB instead of 4 MB of traffic.
    qc_r = qcT.rearrange("(mt p) t -> p mt t", p=128)
    kT_r = kT.rearrange("h d j -> d h j")
    v_r = vb.rearrange("b p a -> p b a")
    wo_r = woT.rearrange("(h d) o -> d h o", d=128)
    # SP: early qc chunks, then wq/v/kT tails ordered by first use
    nc.sync.dma_start(out=qc_sb[:, 0, :], in_=qc_r[:, 0, :])
    nc.sync.dma_start(out=qc_sb[:, 1, :], in_=qc_r[:, 1, :])
    nc.sync.dma_start(out=qc_sb[:, 2:4, :], in_=qc_r[:, 2:4, :])
    nc.sync.dma_start(out=v_sb[:, 3:6, :], in_=v_r[:, 3:6, :])
    nc.sync.dma_start(out=wq_sb[:, 1, :, :], in_=wqT[:, 1, :, :])
    nc.sync.dma_start(out=wq_sb[:, 2, :, :], in_=wqT[:, 2, :, :])
    nc.sync.dma_start(out=v_sb[:, 6:10, :], in_=v_r[:, 6:10, :])
    nc.sync.dma_start(out=wq_sb[:, 3:8, :, :], in_=wqT[:, 3:8, :, :])
    nc.sync.dma_start(out=kT_sb[:, 4:8, :], in_=kT_r[:, 4:8, :])
    nc.sync.dma_start(out=wo_sb, in_=wo_r)
    # ACT: rest of qc, then free for exp
    nc.scalar.dma_start(out=qc_sb[:, 4:6, :], in_=qc_r[:, 4:6, :])
    nc.scalar.dma_start(out=qc_sb[:, 6:8, :], in_=qc_r[:, 6:8, :])
    # Pool: only the head-0 attention streams — it must be free for the
    # partition-reduce denominators by the time attention starts
    nc.gpsimd.dma_start(out=wq_sb[:, 0, :, :], in_=wqT[:, 0, :, :])
    nc.gpsimd.dma_start(out=kT_sb[:, 0, :], in_=kT_r[:, 0, :])
    nc.gpsimd.dma_start(out=mask_sb, in_=maskE)
    nc.gpsimd.dma_start(out=pb_sb, in_=pb)
    for blk in range(3):
        nc.gpsimd.dma_start(out=v_sb[:, blk, :], in_=v_r[:, blk, :])
    nc.gpsimd.dma_start(out=kT_sb[:, 1:4, :], in_=kT_r[:, 1:4, :])

    def _norm_chain(od, stg, h, half):
        # fold the 3 denominator partials, reciprocate, and scale the PSUM
        # output into att (all DVE).  bf16 partials/denominator round at
        # ~0.4% — far inside the rel-err budget — and run at the DVE's 2x
        # 2-byte rate.  Returned as four closures that the caller interleaves
        # with the NEXT half's mask ops: the DVE is strict-FIFO, so emitting
        # the whole chain at once would head-of-line-block those masks and
        # stall the PE's PV matmuls.
        box = {}

        def fold1():
            with nc.allow_low_precision(reason="bf16 softmax denominator"):
                box["den"] = rc_pool.tile([128, 512], BF16, name="den")
                nc.vector.tensor_tensor(
                    box["den"], stg[:, 0, :], stg[:, 1, :], ADD)

        def fold2():
            with nc.allow_low_precision(reason="bf16 softmax denominator"):
                nc.vector.tensor_tensor(
                    box["den"], box["den"], stg[:, 2, :], ADD)

        def recip():
            with nc.allow_low_precision(reason="bf16 softmax denominator"):
                box["rc"] = rc_pool.tile([128, 512], BF16, name="rc")
                nc.vector.reciprocal(box["rc"], box["den"])

        def mult():
            nc.vector.tensor_tensor(
                att_sb[:, h, half * 512:(half + 1) * 512], od, box["rc"], MUL)

        return [fold1, fold2, recip, mult]

    pending_norm = []
    for h in range(NH):
        # ---- Q projection for head h -> qT_sb[d, h, t] (bf16) ----
        # contraction chunks ordered by DMA arrival (SP: 01/23, ACT: 45/67)
        MT_ORDER = (0, 1, 4, 5, 2, 3, 6, 7)

        def _qproj_nn(qh, nn):
            pp = proj_psum.tile([128, 512], F32)
            for i, mt in enumerate(MT_ORDER):
                nc.tensor.matmul(
                    pp,
                    lhsT=wq_sb[:, qh, mt, :],
                    rhs=qc_sb[:, mt, nn * 512:(nn + 1) * 512],
                    start=(i == 0),
                    stop=(i == 7),
                )
            # one PSUM->SBUF copy per engine so neither FIFO eats both
            if nn == 0:
                nc.scalar.copy(qT_sb[:, qh, 0:512], pp)
            else:
                nc.vector.tensor_copy(qT_sb[:, qh, 512:1024], pp)

        for nn in (0, 1):
            _qproj_nn(h, nn)

        # ---- attention for head h, processed in two 512-column halves ----
        # Scores/exp are key-block-major; exp carries a per-partition pad
        # bias (-1e5 on zero-padded halo rows, plus the row-0 denom seed) so
        # padded keys vanish from both the output and the denominator.  The
        # softmax denominators are cross-partition sums done on the idle
        # GPSIMD engine (partition_all_reduce) into a [jj, t] staging buffer
        # (jj = 2-(kb-qt)), folded and reciprocated on DVE — no PE work.
        # The PSUM out accumulation is query-tile-major so each 128-col
        # piece's start..stop group is contiguous per bank.
        for half in range(2):
            q_lo = half * 4                          # first query tile of half
            od = od_psum.tile([128, 512], F32)       # unnormalized out[d, t]
            stg = stg_pool.tile([128, 3, 512], BF16)
            etiles = {}
            # producer runs 4 steps ahead of the PSUM consumer so the PE
            # never waits on the exp round-trip
            for step in range(7):
                kb = q_lo + step
                if pending_norm and step >= 3:
                    # one op of the previous half's normalize chain per step,
                    # interleaved between this half's masks in the DVE FIFO
                    pending_norm.pop(0)()
                if step < 6:
                    qts = [
                        qt for qt in range(max(kb - 2, 0), min(kb, NQT - 1) + 1)
                        if q_lo <= qt < q_lo + 4
                    ]
                    t0 = qts[0] * 128
                    w = 128 * len(qts)
                    sp = s_psum.tile([128, 384], F32)
                    nc.tensor.matmul(
                        sp[:, 0:w],
                        lhsT=kT_sb[:, h, kb * 128:(kb + 1) * 128],
                        rhs=qT_sb[:, h, t0:t0 + w],
                        start=True,
                        stop=True,
                    )
                    e = e_pool.tile([128, 384], BF16)
                    nc.scalar.activation(
                        e[:, 0:w], sp[:, 0:w],
                        mybir.ActivationFunctionType.Exp,
                        bias=pb_sb[:, kb:kb + 1],
                    )
                    # triangular window masks on edge pieces (in-place, DVE)
                    for qt in qts:
                        off = qt * 128 - t0
                        if qt == kb:        # newest keys: valid iff kk >= tt
                            nc.vector.tensor_tensor(
                                e[:, off:off + 128], e[:, off:off + 128],
                                mask_sb[:, 1, :], MUL)
                        elif qt == kb - 2:  # oldest keys: valid iff kk < tt
                            nc.vector.tensor_tensor(
                                e[:, off:off + 128], e[:, off:off + 128],
                                mask_sb[:, 0, :], MUL)
                    # denominator partials: sum e over partitions per piece
                    for qt in qts:
                        off = qt * 128 - t0
                        jj = 2 - (kb - qt)
                        col = qt * 128 - half * 512
                        nc.gpsimd.partition_all_reduce(
                            stg[:, jj, col:col + 128],
                            e[:, off:off + 128],
                            channels=128,
                            reduce_op=bass_isa.ReduceOp.add,
                        )
                    etiles[kb] = (e, t0)
                qt = q_lo + step - 3
                if q_lo <= qt < q_lo + 4:
                    col = qt * 128 - half * 512
                    for j, kbb in enumerate((qt, qt + 1, qt + 2)):
                        e, t0 = etiles[kbb]
                        off = qt * 128 - t0
                        nc.tensor.matmul(
                            od[:, col:col + 128],
                            lhsT=v_sb[:, kbb, h * 128:(h + 1) * 128],
                            rhs=e[:, off:off + 128],
                            start=(j == 0),
                            stop=(j == 2),
                        )
            pending_norm = _norm_chain(od, stg, h, half)
    for op in pending_norm:
        op()

    # ---- output projection: out[t, m] = sum_h att_h[:, t]^T @ Wo_h ----
    # the very last chunk is narrowed so the final copy+DMA drain is short
    for qt in range(NQT):
        chunks = ((0, 512), (512, 1024)) if qt < NQT - 1 else \
                 ((0, 512), (512, 768), (768, 1024))
        for m0, m1 in chunks:
            pp = proj_psum.tile([128, 512], F32)
            for hh in range(NH):
                nc.tensor.matmul(
                    pp[:, 0:m1 - m0],
                    lhsT=att_sb[:, hh, qt * 128:(qt + 1) * 128],
                    rhs=wo_sb[:, hh, m0:m1],
                    start=(hh == 0),
                    stop=(hh == NH - 1),
                )
            st = stage_pool.tile([128, 512], BF16)
            nc.scalar.copy(st[:, 0:m1 - m0], pp[:, 0:m1 - m0])
            nc.sync.dma_start(
                out=out[qt * 128:(qt + 1) * 128, m0:m1],
                in_=st[:, 0:m1 - m0],
            )


_CACHED_NC = {}


def _build_program(iters: int = 1):
    if iters in _CACHED_NC:
        return _CACHED_NC[iters]
    nc = bacc.Bacc("TRN2", target_bir_lowering=False, debug=False)
    qcT = nc.dram_tensor("qcT", [MD, C], BF16, kind="ExternalInput").ap()
    # wqT is pre-laid-out host-side in device order: [p, head, mt, a]
    wqT = nc.dram_tensor("wqT", [128, NH, 8, 128], BF16, kind="ExternalInput").ap()
    woT = nc.dram_tensor("woT", [AD, MD], BF16, kind="ExternalInput").ap()
    kT = nc.dram_tensor("kT", [NH, HD, HALO], BF16, kind="ExternalInput").ap()
    vb = nc.dram_tensor("vb", [NKB, 128, AD], BF16, kind="ExternalInput").ap()
    pb = nc.dram_tensor("pb", [128, NKB], F32, kind="ExternalInput").ap()
    maskE = nc.dram_tensor("maskE", [128, 2, 128], BF16, kind="ExternalInput").ap()
    out = nc.dram_tensor("out", [C, MD], BF16, kind="ExternalOutput").ap()
    from contextlib import ExitStack

    with tile.TileContext(nc) as tc:
        for _ in range(iters):
            with ExitStack() as ctx:
                _emit(ctx, tc, qcT, wqT, woT, kT, vb, pb, maskE, out)
    nc.compile()
    _CACHED_NC[iters] = nc
    return nc


# ----------------------------------------------------------------------------
# host-side shard construction
# ----------------------------------------------------------------------------

def _make_in_maps(query_seq, keys_seq, values_seq, Wq, Wo):
    q = np.asarray(query_seq, dtype=np.float32)
    k = np.asarray(keys_seq, dtype=np.float32)
    v = np.asarray(values_seq, dtype=np.float32)
    wq = np.asarray(Wq, dtype=np.float32)
    wo = np.asarray(Wo, dtype=np.float32)

    scale = np.float32(math.sqrt(float(HD)))
    # [m, a] -> device layout [p, head, mt, a128]  (m = mt*128+p, a = h*128+aa)
    wqT = np.ascontiguousarray(
        (wq.T / scale).reshape(8, 128, NH, 128).transpose(1, 2, 0, 3)
    ).astype(BF)
    woT = np.ascontiguousarray(wo.T).astype(BF)

    # triangular window masks, shared by all cores: [kk, {sub2, sub0}, tt]
    kk = np.arange(128)[:, None]
    tt = np.arange(128)[None, :]
    maskE = np.zeros((128, 2, 128), BF)
    maskE[:, 0, :] = (kk < tt).astype(BF)   # oldest-keys piece
    maskE[:, 1, :] = (kk >= tt).astype(BF)  # newest-keys piece

    in_maps = []
    for core in range(NCORES):
        b, ch = divmod(core, S // C)
        s0 = ch * C

        qcT = np.ascontiguousarray(q[b, s0:s0 + C, :].T).astype(BF)   # [MD, C]

        khalo = np.zeros((HALO, AD), np.float32)
        vhalo = np.zeros((HALO, AD), np.float32)
        lo = s0 - WIN
        off = max(0, -lo)
        khalo[off:] = k[b, lo + off:s0 + C, :]
        vhalo[off:] = v[b, lo + off:s0 + C, :]

        kT = np.ascontiguousarray(
            khalo.reshape(HALO, NH, HD).transpose(1, 2, 0)).astype(BF)
        vb = np.ascontiguousarray(vhalo.reshape(NKB, 128, AD)).astype(BF)

        # per-partition exp bias: -1e5 on zero-padded halo rows (removes them
        # from both the output and the denominator), 0 elsewhere; a single
        # seed entry gives the no-valid-key row 0 denom=1 -> out=0.
        pb = np.zeros((128, NKB), np.float32)
        for blk in range(NKB):
            k_abs = s0 - WIN + blk * 128 + np.arange(128)
            pb[:, blk] = np.where(k_abs >= 0, 0.0, -1.0e5)
        if s0 == 0:
            pb[0, 0] = 0.0
        pb = np.ascontiguousarray(pb)

        in_maps.append({
            "qcT": qcT,
            "wqT": wqT,
            "woT": woT,
            "kT": kT,
            "vb": vb,
            "pb": pb,
            "maskE": maskE,
        })
    return in_maps


def _gather(results) -> np.ndarray:
    out = np.empty((B, S, MD), np.float32)
    for core in range(NCORES):
        b, ch = divmod(core, S // C)
        out[b, ch * C:(ch + 1) * C, :] = results[core]["out"].astype(np.float32)
    return out


def _run(in_maps, **kwargs):
    nc = _build_program()
    return run_bass_kernel_spmd(nc, in_maps, list(range(NCORES)), **kwargs)


def kernel(query_seq, keys_seq, values_seq, Wq, Wo, window=WIN, **_unused):
    assert int(window) == WIN, f"kernel hardcodes window={WIN}, got {window}"
    in_maps = _make_in_maps(query_seq, keys_seq, values_seq, Wq, Wo)
    res = _run(in_maps)
    return _gather(res.results)


def kernel_traced(query_seq, keys_seq, values_seq, Wq, Wo, window=WIN, **_unused):
    """Like kernel() but also returns BassKernelResults (profile/exec time)."""
    assert int(window) == WIN
    in_maps = _make_in_maps(query_seq, keys_seq, values_seq, Wq, Wo)
    res = _run(in_maps, trace=True)
    return _gather(res.results), res
